# revision 9
# baseline (speedup 1.0000x reference)
"""GCN (GCNConv + ReLU) message-passing kernel for 8 Trainium2 NeuronCores.

Strategy (dst-sharded graph parallelism):
  - Nodes sharded contiguously across 8 cores (12500 each).
  - Each core computes h'_c = dinv_c * (x_c @ W) for its shard (PE matmul,
    bf16) in 4 row-chunks; each chunk is AllGathered as soon as it is
    written, so per-edge gathers for chunk g start ~80us into the kernel
    instead of waiting for the full exchange.
  - Edges are grouped by (dst core). Per core, edges are laid out in
    (super-block, chunk-group, dst-block) sections padded to 16 slots;
    per-edge messages are fetched with SWDGE dma_gather (int16 indices into
    the 25000-row chunk-group buffers).
  - Segment-sum over dst via TensorE: one-hot selectors (VectorE is_equal of
    windowed dst codes 128*block_in_super+lane against per-block iota tiles)
    contracted with gathered message tiles, accumulating in PSUM per 128-dst
    block.  Sections are 16-granular; tiles straddling two blocks are simply
    visited by both blocks' selectors (the dst code windows disambiguate).
  - Epilogue fuses (psum * dinv[d]) + (dinv[d]^2*h[d] + b), then ReLU.

Host-side work is limited to integer index preprocessing (edge bucketing,
section offsets) and layout/dtype staging; all floating-point math runs on
device.
"""

import math
import sys

import numpy as np

sys.path.insert(0, "/opt/trn_rl_repo")

import ml_dtypes  # noqa: E402

import concourse.bass as bass  # noqa: E402,F401
import concourse.bacc as bacc  # noqa: E402
import concourse.mybir as mybir  # noqa: E402
from concourse import tile  # noqa: E402
from concourse.bass_utils import run_bass_kernel_spmd  # noqa: E402

BF16 = ml_dtypes.bfloat16

# ----- problem constants (hardcoded; kernel.py must be self-contained) -----
N_NODES = 100000
D_IN = 256
D_OUT = 128
N_CORES = 8
G = 4  # AllGather chunks per core == gather index groups


class Cfg:
    """Static, per-compile configuration (identical across cores)."""

    def __init__(self, n_nodes, d_in, d_out, n_cores, idx_range=None,
                 blocks_per_super=8):
        assert n_nodes % n_cores == 0
        self.n_nodes = n_nodes
        self.d_in = d_in
        self.d_out = d_out
        assert d_out == 128, "kernel assumes 128 output features"
        assert d_in % 128 == 0
        self.kchunks = d_in // 128
        self.n_cores = n_cores
        self.ns = n_nodes // n_cores          # nodes per core
        assert self.ns % G == 0
        self.chunk = self.ns // G             # h' rows per AllGather chunk
        self.grp_rows = self.chunk * n_cores  # rows per gathered group buffer
        assert self.grp_rows <= 32768, "int16 gather index range"
        self.nb = math.ceil(self.ns / 128)    # dst blocks per core
        self.bs = blocks_per_super
        self.nsup = math.ceil(self.nb / self.bs)
        self.ns_pad = self.nb * 128
        # filled by preprocessing (shared across cores):
        self.L16 = None        # [nb, G] 16-granular padded section sizes
        self.run_len = None    # [nsup, G] 128-granular run lengths
        self.sec_off = None    # {(b, g): global slot offset}
        self.sup_off = None    # [nsup+1] global slot offset of each super
        self.tot_slots = None

    def blocks_of_super(self, s):
        return range(s * self.bs, min((s + 1) * self.bs, self.nb))


def preprocess(x, edge_index, W, b, cfg: Cfg):
    """Integer/layout-only host prep. Returns per-core input dicts."""
    ns, chunk = cfg.ns, cfg.chunk
    src = np.asarray(edge_index[0], dtype=np.int64)
    dst = np.asarray(edge_index[1], dtype=np.int64)
    x = np.asarray(x, dtype=np.float32)
    W = np.asarray(W, dtype=np.float32)
    b = np.asarray(b, dtype=np.float32)

    core_of = dst // ns
    order = np.argsort(core_of, kind="stable")
    src, dst = src[order], dst[order]
    core_bounds = np.searchsorted(core_of[order], np.arange(cfg.n_cores + 1))

    percore = []
    counts = np.zeros((cfg.n_cores, cfg.nb, G), dtype=np.int64)
    for c in range(cfg.n_cores):
        lo, hi = core_bounds[c], core_bounds[c + 1]
        s_c, d_c = src[lo:hi], dst[lo:hi] - c * ns
        g_c = (s_c % ns) // chunk
        idxv = (s_c // ns) * chunk + (s_c % ns) - g_c * chunk
        blk = d_c // 128
        key = blk * G + g_c
        # sort by (section, src index): ascending gather addresses within a
        # section give the SDMA engines HBM row-buffer locality
        o = np.lexsort((idxv, key))
        s_c, d_c, g_c, idxv, blk = s_c[o], d_c[o], g_c[o], idxv[o], blk[o]
        cnt = np.bincount(key, minlength=cfg.nb * G).reshape(cfg.nb, G)
        counts[c] = cnt
        deg = np.bincount(d_c, minlength=ns) + 1  # + self loop
        percore.append({"d": d_c, "g": g_c, "idx": idxv, "key": key,
                        "deg": deg, "cnt": cnt})

    # Cross-core-uniform padded section sizes (16-granular), runs padded
    # to 128 so each (super, group) gather run starts tile-aligned.
    Lmax = counts.max(axis=0)                       # [nb, G]
    L16 = Lmax.copy()
    cfg.L16 = L16

    run_len = np.zeros((cfg.nsup, G), dtype=np.int64)
    sec_off = {}
    off = 0
    sup_off = [0]
    for s in range(cfg.nsup):
        for g in range(G):
            run_raw = 0
            for bb in cfg.blocks_of_super(s):
                sec_off[(bb, g)] = off + run_raw
                run_raw += int(L16[bb, g])
            rl = ((run_raw + 127) // 128) * 128
            run_len[s, g] = rl
            off += rl
        sup_off.append(off)
    cfg.run_len = run_len
    cfg.sec_off = sec_off
    cfg.sup_off = np.asarray(sup_off, dtype=np.int64)
    tot_slots = int(off)
    cfg.tot_slots = tot_slots
    assert tot_slots % 128 == 0

    in_maps = []
    for c in range(cfg.n_cores):
        pc = percore[c]
        idx_all = np.zeros(tot_slots, dtype=np.int16)
        dst_all = np.full(tot_slots, -1.0, dtype=np.float32)
        # edges sorted by key=(blk, g); place each section's run
        cnt = pc["cnt"]
        starts = np.zeros_like(cnt)
        flat = np.concatenate([[0], np.cumsum(cnt.ravel())])
        for bb in range(cfg.nb):
            s = bb // cfg.bs
            for g in range(G):
                k = bb * G + g
                m = int(cnt[bb, g])
                if m:
                    sl = slice(int(flat[k]), int(flat[k]) + m)
                    o0 = sec_off[(bb, g)]
                    idx_all[o0:o0 + m] = pc["idx"][sl].astype(np.int16)
                    dst_all[o0:o0 + m] = (
                        128 * (bb - s * cfg.bs) + (pc["d"][sl] - bb * 128)
                    ).astype(np.float32)
        # wrap idx into 16 partitions, replicated to 128
        idx_w16 = idx_all.reshape(-1, 16).T.copy()          # [16, tot/16]
        idx_w = np.tile(idx_w16, (8, 1))                     # [128, tot/16]
        dst_w = np.ascontiguousarray(
            dst_all.reshape(-1, 128).T).astype(np.float16)   # [128, tot/128]

        deg_pad = np.ones(cfg.ns_pad, dtype=np.int32)
        deg_pad[:ns] = pc["deg"].astype(np.int32)
        deg_w = np.ascontiguousarray(deg_pad.reshape(cfg.nb, 128).T)

        xs = x[c * ns:(c + 1) * ns]
        xT = np.zeros((cfg.d_in, cfg.ns_pad), dtype=np.float32)
        xT[:, :ns] = xs.T

        in_maps.append({
            "xT": np.ascontiguousarray(xT).astype(BF16),
            "Wm": W.astype(BF16),
            "bias": b.reshape(1, -1).copy(),
            "deg": deg_w,
            "idx": idx_w,
            "dstloc": dst_w,
        })
    return in_maps, tot_slots


def build_program(cfg: Cfg, tot_slots, mock_cc=False, gchunk=8):
    """Builds the SPMD bass program (same NEFF on every core)."""
    fp32 = mybir.dt.float32
    bf16 = mybir.dt.bfloat16
    L16 = cfg.L16
    DO = cfg.d_out
    ns, chunk = cfg.ns, cfg.chunk
    max_idx_per_gather = 1024

    nc = bacc.Bacc("TRN2", target_bir_lowering=False,
                   num_devices=cfg.n_cores, debug=False)

    xT = nc.dram_tensor("xT", [cfg.d_in, cfg.ns_pad], bf16, kind="ExternalInput")
    Wm = nc.dram_tensor("Wm", [cfg.d_in, DO], bf16, kind="ExternalInput")
    bias = nc.dram_tensor("bias", [1, DO], fp32, kind="ExternalInput")
    deg = nc.dram_tensor("deg", [128, cfg.nb], mybir.dt.int32, kind="ExternalInput")
    idx = nc.dram_tensor("idx", [128, tot_slots // 16], mybir.dt.int16,
                         kind="ExternalInput")
    fp16 = mybir.dt.float16
    dstloc = nc.dram_tensor("dstloc", [128, tot_slots // 128], fp16,
                            kind="ExternalInput")
    out = nc.dram_tensor("out", [cfg.ns, DO], fp32, kind="ExternalOutput")

    cc_in = nc.dram_tensor("cc_in", [cfg.ns, DO], bf16)
    cc_out = [nc.dram_tensor(f"cc_out{g}", [cfg.grp_rows, DO], bf16,
                             addr_space="Shared") for g in range(G)]

    # dst-block after which AllGather chunk g can fire
    ag_after_block = [(chunk * (g + 1) + 127) // 128 - 1 for g in range(G)]
    max_sup_tiles = int(cfg.run_len.sum(axis=1).max()) // 128

    from contextlib import ExitStack
    with tile.TileContext(nc) as tc, ExitStack() as ctx:
        const = ctx.enter_context(tc.tile_pool(name="const", bufs=1))
        hpsum = ctx.enter_context(tc.tile_pool(name="hpsum", bufs=2, space="PSUM"))
        htmp = ctx.enter_context(tc.tile_pool(name="htmp", bufs=3))
        msgp = ctx.enter_context(tc.tile_pool(name="msg", bufs=2))
        selp = ctx.enter_context(tc.tile_pool(name="sel", bufs=3))
        pspool = ctx.enter_context(tc.tile_pool(name="ps", bufs=4, space="PSUM"))
        epool = ctx.enter_context(tc.tile_pool(name="ep", bufs=4))

        # ---------------- phase 0: constants ----------------
        W_b = const.tile([128, cfg.kchunks, DO], bf16)
        for k in range(cfg.kchunks):
            nc.sync.dma_start(W_b[:, k, :], Wm[k * 128:(k + 1) * 128, :])

        xb = const.tile([128, cfg.kchunks, cfg.ns_pad], bf16)
        c0_cols = min(cfg.ns_pad, ((chunk + 127) // 128) * 128)
        for k in range(cfg.kchunks):
            nc.sync.dma_start(xb[:, k, :c0_cols],
                              xT[k * 128:(k + 1) * 128, :c0_cols])
        for k in range(cfg.kchunks):
            nc.sync.dma_start(xb[:, k, c0_cols:],
                              xT[k * 128:(k + 1) * 128, c0_cols:])

        deg_i = const.tile([128, cfg.nb], mybir.dt.int32)
        nc.sync.dma_start(deg_i[:, :], deg[:, :])
        deg_f = const.tile([128, cfg.nb], fp32)
        nc.vector.tensor_copy(deg_f[:, :], deg_i[:, :])
        deg_sq = const.tile([128, cfg.nb], fp32)
        nc.scalar.activation(deg_sq[:, :], deg_f[:, :],
                             mybir.ActivationFunctionType.Sqrt)
        dinv = const.tile([128, cfg.nb], fp32)
        nc.vector.reciprocal(dinv[:, :], deg_sq[:, :])
        dinv2 = const.tile([128, cfg.nb], fp32)
        nc.vector.tensor_tensor(dinv2[:, :], dinv[:, :], dinv[:, :],
                                mybir.AluOpType.mult)

        b_row = const.tile([1, DO], fp32)
        nc.sync.dma_start(b_row[:, :], bias[:, :])
        ones_row = const.tile([1, 128], fp32)
        nc.vector.memset(ones_row[:, :], 1.0)
        bt_ps = hpsum.tile([128, DO], fp32, tag="hps")
        nc.tensor.matmul(bt_ps[:, :], ones_row[:, :], b_row[:, :],
                         start=True, stop=True)
        b_tile = const.tile([128, DO], fp32)
        nc.vector.tensor_copy(b_tile[:, :], bt_ps[:, :])

        # per-block-in-super iota tiles: values 128*bb + [0..127]
        iota0 = const.tile([128, 128], fp32)
        nc.gpsimd.iota(iota0[:, :], [[1, 128]], channel_multiplier=0,
                       allow_small_or_imprecise_dtypes=True)
        iota_b = const.tile([128, cfg.bs, 128], fp16)
        for bb in range(cfg.bs):
            nc.vector.tensor_scalar(iota_b[:, bb, :], iota0[:, :],
                                    float(128 * bb), None,
                                    mybir.AluOpType.add)

        sel_tiles_max = 1
        for bb in range(cfg.nb):
            s = bb // cfg.bs
            sup0 = int(cfg.sup_off[s])
            nt = 0
            for g in range(G):
                ln = int(L16[bb, g])
                if ln == 0:
                    continue
                o = cfg.sec_off[(bb, g)]
                nt += (o + ln - 1 - sup0) // 128 - (o - sup0) // 128 + 1
            sel_tiles_max = max(sel_tiles_max, nt)

        # ---------------- phase A: h' = dinv * (x @ W) -------------------
        # h' blocks staged in SBUF; each AllGather chunk written to cc_in
        # with one large DMA (per-block writes pay ~2us HWDGE issue each).
        hstage = const.tile([128, cfg.nb, DO], bf16)
        bnds = sorted({min(cfg.ns, ((chunk * (g + 1) + 127) // 128) * 128)
                       for g in range(G)})
        dma_after_block = {(hi + 127) // 128 - 1: (lo, hi)
                           for lo, hi in zip([0] + bnds[:-1], bnds)}
        for k in range(cfg.nb):
            ph = hpsum.tile([128, DO], fp32, tag="hps")
            for kb in range(cfg.kchunks):
                nc.tensor.matmul(ph[:, :], xb[:, kb, k * 128:(k + 1) * 128],
                                 W_b[:, kb, :],
                                 start=(kb == 0), stop=(kb == cfg.kchunks - 1))
            nc.vector.tensor_scalar(hstage[:, k, :], ph[:, :],
                                    dinv[:, k:k + 1], None,
                                    mybir.AluOpType.mult)
            if k in dma_after_block:
                lo, hi = dma_after_block[k]
                full_hi = (hi // 128) * 128
                if full_hi > lo:
                    nc.sync.dma_start(
                        cc_in[lo:full_hi, :].rearrange(
                            "(t p) d -> p t d", p=128),
                        hstage[:, lo // 128:full_hi // 128, :])
                if hi > full_hi:
                    nc.sync.dma_start(cc_in[full_hi:hi, :],
                                      hstage[:hi - full_hi, hi // 128, :])


        idx_sb = const.tile([128, tot_slots // 16], mybir.dt.int16)
        nc.sync.dma_start(idx_sb[:, :], idx[:, :])
        dst_sb = const.tile([128, tot_slots // 128], fp16)
        nc.sync.dma_start(dst_sb[:, :], dstloc[:, :])

        # ---------------- phase C: gather + segment-sum + epilogue -------
        ag_done = set()

        def dispatch_ag(g):
            ag_done.add(g)
            lo, hi = chunk * g, chunk * (g + 1)
            if cfg.n_cores > 1 and not mock_cc:
                nc.gpsimd.collective_compute(
                    "AllGather",
                    mybir.AluOpType.bypass,
                    replica_groups=[list(range(cfg.n_cores))],
                    ins=[cc_in[lo:hi, :]],
                    outs=[cc_out[g][:, :]],
                )
            else:
                for cpy in range(cfg.n_cores):
                    nc.sync.dma_start(
                        cc_out[g][cpy * chunk:(cpy + 1) * chunk, :],
                        cc_in[lo:hi, :])

        for s in range(cfg.nsup):
            nt_sup = int(cfg.run_len[s].sum()) // 128
            if nt_sup == 0:
                continue
            sup0 = int(cfg.sup_off[s])
            msg = msgp.tile([128, max_sup_tiles, 128], bf16, tag="msg")
            for g in range(G):
                rl = int(cfg.run_len[s, g])
                if rl == 0:
                    continue
                if g not in ag_done:
                    dispatch_ag(g)
                run0 = cfg.sec_off[(s * cfg.bs, g)]  # first block's section
                t0 = (run0 - sup0) // 128
                for c0 in range(0, rl, max_idx_per_gather):
                    n_sg = min(max_idx_per_gather, rl - c0)
                    slot0 = run0 + c0
                    nc.gpsimd.dma_gather(
                        msg[:, t0 + c0 // 128:t0 + (c0 + n_sg) // 128, :],
                        cc_out[g][:, :],
                        idx_sb[:, slot0 // 16:(slot0 + n_sg) // 16],
                        n_sg, n_sg, DO)

            for bb in cfg.blocks_of_super(s):
                lb = bb - s * cfg.bs
                ps = pspool.tile([128, DO], fp32, tag="ps")
                # (tile, g) pairs this block's sections touch
                tiles = []
                for g in range(G):
                    o = cfg.sec_off[(bb, g)]
                    ln = int(L16[bb, g])
                    if ln == 0:
                        continue
                    ta = (o - sup0) // 128
                    tz = (o + ln - 1 - sup0) // 128
                    tiles.extend(range(ta, tz + 1))
                tiles = sorted(set(tiles))
                if not tiles:
                    t1 = epool.tile([128, DO], fp32, tag="t1")
                    nc.vector.scalar_tensor_tensor(
                        t1[:, :], hstage[:, bb, :], dinv[:, bb:bb + 1],
                        b_tile[:, :],
                        mybir.AluOpType.mult, mybir.AluOpType.add)
                    t2 = epool.tile([128, DO], fp32, tag="t2")
                    nc.scalar.activation(t2[:, :], t1[:, :],
                                         mybir.ActivationFunctionType.Relu)
                    rows = min(128, cfg.ns - bb * 128)
                    nc.sync.dma_start(out[bb * 128:bb * 128 + rows, :],
                                      t2[:rows, :])
                    continue
                sel = selp.tile([128, sel_tiles_max, 128], bf16, tag="sel")
                for j, t in enumerate(tiles):
                    d_ap = dst_sb[:, sup0 // 128 + t:sup0 // 128 + t + 1]
                    d_b = d_ap.broadcast_to((128, 128))
                    nc.vector.tensor_tensor(sel[:, j, :], d_b,
                                            iota_b[:, lb, :],
                                            mybir.AluOpType.is_equal)
                    nc.tensor.matmul(
                        ps[:, :], sel[:, j, :], msg[:, t, :],
                        start=(j == 0), stop=(j == len(tiles) - 1))
                t0 = epool.tile([128, DO], fp32, tag="t0")
                nc.vector.scalar_tensor_tensor(
                    t0[:, :], hstage[:, bb, :], dinv[:, bb:bb + 1],
                    b_tile[:, :],
                    mybir.AluOpType.mult, mybir.AluOpType.add)
                t1 = epool.tile([128, DO], fp32, tag="t1")
                nc.vector.scalar_tensor_tensor(
                    t1[:, :], ps[:, :], dinv[:, bb:bb + 1], t0[:, :],
                    mybir.AluOpType.mult, mybir.AluOpType.add)
                t2 = epool.tile([128, DO], fp32, tag="t2")
                nc.scalar.activation(t2[:, :], t1[:, :],
                                     mybir.ActivationFunctionType.Relu)
                rows = min(128, cfg.ns - bb * 128)
                nc.sync.dma_start(out[bb * 128:bb * 128 + rows, :],
                                  t2[:rows, :])

    nc.compile()
    return nc


def kernel(x, edge_index, W, b):
    cfg = Cfg(N_NODES, D_IN, D_OUT, N_CORES)
    in_maps, tot_slots = preprocess(x, edge_index, W, b, cfg)
    nc = build_program(cfg, tot_slots)
    res = run_bass_kernel_spmd(nc, in_maps, list(range(N_CORES)))
    outs = [r["out"][:cfg.ns] for r in res.results]
    return np.concatenate(outs, axis=0).astype(np.float32)


if __name__ == "__main__":
    cfg = Cfg(N_NODES, D_IN, D_OUT, N_CORES)
    print("cfg", cfg.nb, cfg.nsup, cfg.chunk)


# revision 10
# speedup vs baseline: 1.0081x; 1.0081x over previous
"""GCN (GCNConv + ReLU) message-passing kernel for 8 Trainium2 NeuronCores.

Strategy (dst-sharded graph parallelism):
  - Nodes sharded contiguously across 8 cores (12500 each).
  - Each core computes h'_c = dinv_c * (x_c @ W) for its shard (PE matmul,
    bf16) in 4 row-chunks; each chunk is AllGathered as soon as it is
    written, so per-edge gathers for chunk g start ~80us into the kernel
    instead of waiting for the full exchange.
  - Edges are grouped by (dst core). Per core, edges are laid out in
    (super-block, chunk-group, dst-block) sections padded to 16 slots;
    per-edge messages are fetched with SWDGE dma_gather (int16 indices into
    the 25000-row chunk-group buffers).
  - Segment-sum over dst via TensorE: one-hot selectors (VectorE is_equal of
    windowed dst codes 128*block_in_super+lane against per-block iota tiles)
    contracted with gathered message tiles, accumulating in PSUM per 128-dst
    block.  Sections are 16-granular; tiles straddling two blocks are simply
    visited by both blocks' selectors (the dst code windows disambiguate).
  - Epilogue fuses (psum * dinv[d]) + (dinv[d]^2*h[d] + b), then ReLU.

Host-side work is limited to integer index preprocessing (edge bucketing,
section offsets) and layout/dtype staging; all floating-point math runs on
device.
"""

import math
import sys

import numpy as np

sys.path.insert(0, "/opt/trn_rl_repo")

import ml_dtypes  # noqa: E402

import concourse.bass as bass  # noqa: E402,F401
import concourse.bacc as bacc  # noqa: E402
import concourse.mybir as mybir  # noqa: E402
from concourse import tile  # noqa: E402
from concourse.bass_utils import run_bass_kernel_spmd  # noqa: E402

BF16 = ml_dtypes.bfloat16

# ----- problem constants (hardcoded; kernel.py must be self-contained) -----
N_NODES = 100000
D_IN = 256
D_OUT = 128
N_CORES = 8
G = 4  # AllGather chunks per core == gather index groups


class Cfg:
    """Static, per-compile configuration (identical across cores)."""

    def __init__(self, n_nodes, d_in, d_out, n_cores, idx_range=None,
                 blocks_per_super=8):
        assert n_nodes % n_cores == 0
        self.n_nodes = n_nodes
        self.d_in = d_in
        self.d_out = d_out
        assert d_out == 128, "kernel assumes 128 output features"
        assert d_in % 128 == 0
        self.kchunks = d_in // 128
        self.n_cores = n_cores
        self.ns = n_nodes // n_cores          # nodes per core
        assert self.ns % G == 0
        self.chunk = self.ns // G             # h' rows per AllGather chunk
        self.grp_rows = self.chunk * n_cores  # rows per gathered group buffer
        assert self.grp_rows <= 32768, "int16 gather index range"
        self.nb = math.ceil(self.ns / 128)    # dst blocks per core
        self.bs = blocks_per_super
        self.nsup = math.ceil(self.nb / self.bs)
        self.ns_pad = self.nb * 128
        # filled by preprocessing (shared across cores):
        self.L16 = None        # [nb, G] 16-granular padded section sizes
        self.run_len = None    # [nsup, G] 128-granular run lengths
        self.sec_off = None    # {(b, g): global slot offset}
        self.sup_off = None    # [nsup+1] global slot offset of each super
        self.tot_slots = None

    def blocks_of_super(self, s):
        return range(s * self.bs, min((s + 1) * self.bs, self.nb))


def preprocess(x, edge_index, W, b, cfg: Cfg):
    """Integer/layout-only host prep. Returns per-core input dicts."""
    ns, chunk = cfg.ns, cfg.chunk
    src = np.asarray(edge_index[0], dtype=np.int64)
    dst = np.asarray(edge_index[1], dtype=np.int64)
    x = np.asarray(x, dtype=np.float32)
    W = np.asarray(W, dtype=np.float32)
    b = np.asarray(b, dtype=np.float32)

    core_of = dst // ns
    order = np.argsort(core_of, kind="stable")
    src, dst = src[order], dst[order]
    core_bounds = np.searchsorted(core_of[order], np.arange(cfg.n_cores + 1))

    percore = []
    counts = np.zeros((cfg.n_cores, cfg.nb, G), dtype=np.int64)
    for c in range(cfg.n_cores):
        lo, hi = core_bounds[c], core_bounds[c + 1]
        s_c, d_c = src[lo:hi], dst[lo:hi] - c * ns
        g_c = (s_c % ns) // chunk
        idxv = (s_c // ns) * chunk + (s_c % ns) - g_c * chunk
        blk = d_c // 128
        key = blk * G + g_c
        # sort by (section, src index): ascending gather addresses within a
        # section give the SDMA engines HBM row-buffer locality
        o = np.lexsort((idxv, key))
        s_c, d_c, g_c, idxv, blk = s_c[o], d_c[o], g_c[o], idxv[o], blk[o]
        cnt = np.bincount(key, minlength=cfg.nb * G).reshape(cfg.nb, G)
        counts[c] = cnt
        deg = np.bincount(d_c, minlength=ns) + 1  # + self loop
        percore.append({"d": d_c, "g": g_c, "idx": idxv, "key": key,
                        "deg": deg, "cnt": cnt})

    # Cross-core-uniform padded section sizes (16-granular), runs padded
    # to 128 so each (super, group) gather run starts tile-aligned.
    Lmax = counts.max(axis=0)                       # [nb, G]
    L16 = Lmax.copy()
    cfg.L16 = L16

    run_len = np.zeros((cfg.nsup, G), dtype=np.int64)
    sec_off = {}
    off = 0
    sup_off = [0]
    for s in range(cfg.nsup):
        for g in range(G):
            run_raw = 0
            for bb in cfg.blocks_of_super(s):
                sec_off[(bb, g)] = off + run_raw
                run_raw += int(L16[bb, g])
            rl = ((run_raw + 127) // 128) * 128
            run_len[s, g] = rl
            off += rl
        sup_off.append(off)
    cfg.run_len = run_len
    cfg.sec_off = sec_off
    cfg.sup_off = np.asarray(sup_off, dtype=np.int64)
    tot_slots = int(off)
    cfg.tot_slots = tot_slots
    assert tot_slots % 128 == 0

    in_maps = []
    for c in range(cfg.n_cores):
        pc = percore[c]
        idx_all = np.zeros(tot_slots, dtype=np.int16)
        dst_all = np.full(tot_slots, -1.0, dtype=np.float32)
        # edges sorted by key=(blk, g); place each section's run
        cnt = pc["cnt"]
        starts = np.zeros_like(cnt)
        flat = np.concatenate([[0], np.cumsum(cnt.ravel())])
        for bb in range(cfg.nb):
            s = bb // cfg.bs
            for g in range(G):
                k = bb * G + g
                m = int(cnt[bb, g])
                if m:
                    sl = slice(int(flat[k]), int(flat[k]) + m)
                    o0 = sec_off[(bb, g)]
                    idx_all[o0:o0 + m] = pc["idx"][sl].astype(np.int16)
                    dst_all[o0:o0 + m] = (
                        128 * (bb - s * cfg.bs) + (pc["d"][sl] - bb * 128)
                    ).astype(np.float32)
        # wrap idx into 16 partitions, replicated to 128
        idx_w16 = idx_all.reshape(-1, 16).T.copy()          # [16, tot/16]
        idx_w = np.tile(idx_w16, (8, 1))                     # [128, tot/16]
        dst_w = np.ascontiguousarray(
            dst_all.reshape(-1, 128).T).astype(np.float16)   # [128, tot/128]

        deg_pad = np.ones(cfg.ns_pad, dtype=np.int32)
        deg_pad[:ns] = pc["deg"].astype(np.int32)
        deg_w = np.ascontiguousarray(deg_pad.reshape(cfg.nb, 128).T)

        xs = x[c * ns:(c + 1) * ns]
        xT = np.zeros((cfg.d_in, cfg.ns_pad), dtype=np.float32)
        xT[:, :ns] = xs.T

        in_maps.append({
            "xT": np.ascontiguousarray(xT).astype(BF16),
            "Wm": W.astype(BF16),
            "bias": b.reshape(1, -1).copy(),
            "deg": deg_w,
            "idx": idx_w,
            "dstloc": dst_w,
        })
    return in_maps, tot_slots


def build_program(cfg: Cfg, tot_slots, mock_cc=False, gchunk=8):
    """Builds the SPMD bass program (same NEFF on every core)."""
    fp32 = mybir.dt.float32
    bf16 = mybir.dt.bfloat16
    L16 = cfg.L16
    DO = cfg.d_out
    ns, chunk = cfg.ns, cfg.chunk
    max_idx_per_gather = 1024

    nc = bacc.Bacc("TRN2", target_bir_lowering=False,
                   num_devices=cfg.n_cores, debug=False)

    xT = nc.dram_tensor("xT", [cfg.d_in, cfg.ns_pad], bf16, kind="ExternalInput")
    Wm = nc.dram_tensor("Wm", [cfg.d_in, DO], bf16, kind="ExternalInput")
    bias = nc.dram_tensor("bias", [1, DO], fp32, kind="ExternalInput")
    deg = nc.dram_tensor("deg", [128, cfg.nb], mybir.dt.int32, kind="ExternalInput")
    idx = nc.dram_tensor("idx", [128, tot_slots // 16], mybir.dt.int16,
                         kind="ExternalInput")
    fp16 = mybir.dt.float16
    dstloc = nc.dram_tensor("dstloc", [128, tot_slots // 128], fp16,
                            kind="ExternalInput")
    out = nc.dram_tensor("out", [cfg.ns, DO], fp32, kind="ExternalOutput")

    cc_in = nc.dram_tensor("cc_in", [cfg.ns, DO], bf16)
    cc_out = [nc.dram_tensor(f"cc_out{g}", [cfg.grp_rows, DO], bf16,
                             addr_space="Shared") for g in range(G)]

    # dst-block after which AllGather chunk g can fire
    ag_after_block = [(chunk * (g + 1) + 127) // 128 - 1 for g in range(G)]
    max_sup_tiles = int(cfg.run_len.sum(axis=1).max()) // 128

    from contextlib import ExitStack
    with tile.TileContext(nc) as tc, ExitStack() as ctx:
        const = ctx.enter_context(tc.tile_pool(name="const", bufs=1))
        hpsum = ctx.enter_context(tc.tile_pool(name="hpsum", bufs=2, space="PSUM"))
        htmp = ctx.enter_context(tc.tile_pool(name="htmp", bufs=3))
        msgp = ctx.enter_context(tc.tile_pool(name="msg", bufs=2))
        selp = ctx.enter_context(tc.tile_pool(name="sel", bufs=3))
        pspool = ctx.enter_context(tc.tile_pool(name="ps", bufs=4, space="PSUM"))
        epool = ctx.enter_context(tc.tile_pool(name="ep", bufs=4))

        # ---------------- phase 0: constants ----------------
        W_b = const.tile([128, cfg.kchunks, DO], bf16)
        for k in range(cfg.kchunks):
            nc.sync.dma_start(W_b[:, k, :], Wm[k * 128:(k + 1) * 128, :])

        xb = const.tile([128, cfg.kchunks, cfg.ns_pad], bf16)
        c0_cols = min(cfg.ns_pad, ((chunk + 127) // 128) * 128)
        for k in range(cfg.kchunks):
            nc.sync.dma_start(xb[:, k, :c0_cols],
                              xT[k * 128:(k + 1) * 128, :c0_cols])
        for k in range(cfg.kchunks):
            nc.sync.dma_start(xb[:, k, c0_cols:],
                              xT[k * 128:(k + 1) * 128, c0_cols:])

        deg_i = const.tile([128, cfg.nb], mybir.dt.int32)
        nc.sync.dma_start(deg_i[:, :], deg[:, :])
        deg_f = const.tile([128, cfg.nb], fp32)
        nc.vector.tensor_copy(deg_f[:, :], deg_i[:, :])
        deg_sq = const.tile([128, cfg.nb], fp32)
        nc.scalar.activation(deg_sq[:, :], deg_f[:, :],
                             mybir.ActivationFunctionType.Sqrt)
        dinv = const.tile([128, cfg.nb], fp32)
        nc.vector.reciprocal(dinv[:, :], deg_sq[:, :])
        dinv2 = const.tile([128, cfg.nb], fp32)
        nc.vector.tensor_tensor(dinv2[:, :], dinv[:, :], dinv[:, :],
                                mybir.AluOpType.mult)

        b_row = const.tile([1, DO], fp32)
        nc.sync.dma_start(b_row[:, :], bias[:, :])
        ones_row = const.tile([1, 128], fp32)
        nc.vector.memset(ones_row[:, :], 1.0)
        bt_ps = hpsum.tile([128, DO], fp32, tag="hps")
        nc.tensor.matmul(bt_ps[:, :], ones_row[:, :], b_row[:, :],
                         start=True, stop=True)
        b_tile = const.tile([128, DO], fp32)
        nc.vector.tensor_copy(b_tile[:, :], bt_ps[:, :])

        # per-block-in-super iota tiles: values 128*bb + [0..127]
        iota0 = const.tile([128, 128], fp32)
        nc.gpsimd.iota(iota0[:, :], [[1, 128]], channel_multiplier=0,
                       allow_small_or_imprecise_dtypes=True)
        iota_b = const.tile([128, cfg.bs, 128], fp16)
        for bb in range(cfg.bs):
            nc.vector.tensor_scalar(iota_b[:, bb, :], iota0[:, :],
                                    float(128 * bb), None,
                                    mybir.AluOpType.add)

        sel_tiles_max = 1
        for bb in range(cfg.nb):
            s = bb // cfg.bs
            sup0 = int(cfg.sup_off[s])
            nt = 0
            for g in range(G):
                ln = int(L16[bb, g])
                if ln == 0:
                    continue
                o = cfg.sec_off[(bb, g)]
                nt += (o + ln - 1 - sup0) // 128 - (o - sup0) // 128 + 1
            sel_tiles_max = max(sel_tiles_max, nt)

        # ---------------- phase A: h' = dinv * (x @ W) -------------------
        # h' blocks staged in SBUF; each AllGather chunk written to cc_in
        # with one large DMA (per-block writes pay ~2us HWDGE issue each).
        hstage = const.tile([128, cfg.nb, DO], bf16)
        bnds = sorted({min(cfg.ns, ((chunk * (g + 1) + 127) // 128) * 128)
                       for g in range(G)})
        dma_after_block = {(hi + 127) // 128 - 1: (lo, hi)
                           for lo, hi in zip([0] + bnds[:-1], bnds)}
        for k in range(cfg.nb):
            ph = hpsum.tile([128, DO], fp32, tag="hps")
            for kb in range(cfg.kchunks):
                nc.tensor.matmul(ph[:, :], xb[:, kb, k * 128:(k + 1) * 128],
                                 W_b[:, kb, :],
                                 start=(kb == 0), stop=(kb == cfg.kchunks - 1))
            nc.vector.tensor_scalar(hstage[:, k, :], ph[:, :],
                                    dinv[:, k:k + 1], None,
                                    mybir.AluOpType.mult)
            if k in dma_after_block:
                lo, hi = dma_after_block[k]
                full_hi = (hi // 128) * 128
                if full_hi > lo:
                    nc.sync.dma_start(
                        cc_in[lo:full_hi, :].rearrange(
                            "(t p) d -> p t d", p=128),
                        hstage[:, lo // 128:full_hi // 128, :])
                if hi > full_hi:
                    nc.sync.dma_start(cc_in[full_hi:hi, :],
                                      hstage[:hi - full_hi, hi // 128, :])


        idx_sb = const.tile([128, tot_slots // 16], mybir.dt.int16)
        nc.sync.dma_start(idx_sb[:, :], idx[:, :])
        dst_sb = const.tile([128, tot_slots // 128], fp16)
        nc.sync.dma_start(dst_sb[:, :], dstloc[:, :])

        # ---------------- phase C: gather + segment-sum + epilogue -------
        ag_done = set()

        def dispatch_ag(g):
            ag_done.add(g)
            lo, hi = chunk * g, chunk * (g + 1)
            if cfg.n_cores > 1 and not mock_cc:
                nc.gpsimd.collective_compute(
                    "AllGather",
                    mybir.AluOpType.bypass,
                    replica_groups=[list(range(cfg.n_cores))],
                    ins=[cc_in[lo:hi, :]],
                    outs=[cc_out[g][:, :]],
                )
            else:
                for cpy in range(cfg.n_cores):
                    nc.sync.dma_start(
                        cc_out[g][cpy * chunk:(cpy + 1) * chunk, :],
                        cc_in[lo:hi, :])

        for s in range(cfg.nsup):
            nt_sup = int(cfg.run_len[s].sum()) // 128
            if nt_sup == 0:
                continue
            sup0 = int(cfg.sup_off[s])
            msg = msgp.tile([128, max_sup_tiles, 128], bf16, tag="msg")
            for g in range(G):
                rl = int(cfg.run_len[s, g])
                if rl == 0:
                    continue
                if g not in ag_done:
                    dispatch_ag(g)
                run0 = cfg.sec_off[(s * cfg.bs, g)]  # first block's section
                t0 = (run0 - sup0) // 128
                for c0 in range(0, rl, max_idx_per_gather):
                    n_sg = min(max_idx_per_gather, rl - c0)
                    slot0 = run0 + c0
                    nc.gpsimd.dma_gather(
                        msg[:, t0 + c0 // 128:t0 + (c0 + n_sg) // 128, :],
                        cc_out[g][:, :],
                        idx_sb[:, slot0 // 16:(slot0 + n_sg) // 16],
                        n_sg, n_sg, DO, single_packet=False)

            for bb in cfg.blocks_of_super(s):
                lb = bb - s * cfg.bs
                ps = pspool.tile([128, DO], fp32, tag="ps")
                # (tile, g) pairs this block's sections touch
                tiles = []
                for g in range(G):
                    o = cfg.sec_off[(bb, g)]
                    ln = int(L16[bb, g])
                    if ln == 0:
                        continue
                    ta = (o - sup0) // 128
                    tz = (o + ln - 1 - sup0) // 128
                    tiles.extend(range(ta, tz + 1))
                tiles = sorted(set(tiles))
                if not tiles:
                    t1 = epool.tile([128, DO], fp32, tag="t1")
                    nc.vector.scalar_tensor_tensor(
                        t1[:, :], hstage[:, bb, :], dinv[:, bb:bb + 1],
                        b_tile[:, :],
                        mybir.AluOpType.mult, mybir.AluOpType.add)
                    t2 = epool.tile([128, DO], fp32, tag="t2")
                    nc.scalar.activation(t2[:, :], t1[:, :],
                                         mybir.ActivationFunctionType.Relu)
                    rows = min(128, cfg.ns - bb * 128)
                    nc.sync.dma_start(out[bb * 128:bb * 128 + rows, :],
                                      t2[:rows, :])
                    continue
                sel = selp.tile([128, sel_tiles_max, 128], bf16, tag="sel")
                for j, t in enumerate(tiles):
                    d_ap = dst_sb[:, sup0 // 128 + t:sup0 // 128 + t + 1]
                    d_b = d_ap.broadcast_to((128, 128))
                    nc.vector.tensor_tensor(sel[:, j, :], d_b,
                                            iota_b[:, lb, :],
                                            mybir.AluOpType.is_equal)
                    nc.tensor.matmul(
                        ps[:, :], sel[:, j, :], msg[:, t, :],
                        start=(j == 0), stop=(j == len(tiles) - 1))
                t0 = epool.tile([128, DO], fp32, tag="t0")
                nc.vector.scalar_tensor_tensor(
                    t0[:, :], hstage[:, bb, :], dinv[:, bb:bb + 1],
                    b_tile[:, :],
                    mybir.AluOpType.mult, mybir.AluOpType.add)
                t1 = epool.tile([128, DO], fp32, tag="t1")
                nc.vector.scalar_tensor_tensor(
                    t1[:, :], ps[:, :], dinv[:, bb:bb + 1], t0[:, :],
                    mybir.AluOpType.mult, mybir.AluOpType.add)
                t2 = epool.tile([128, DO], fp32, tag="t2")
                nc.scalar.activation(t2[:, :], t1[:, :],
                                     mybir.ActivationFunctionType.Relu)
                rows = min(128, cfg.ns - bb * 128)
                nc.sync.dma_start(out[bb * 128:bb * 128 + rows, :],
                                  t2[:rows, :])

    nc.compile()
    return nc


def kernel(x, edge_index, W, b):
    cfg = Cfg(N_NODES, D_IN, D_OUT, N_CORES)
    in_maps, tot_slots = preprocess(x, edge_index, W, b, cfg)
    nc = build_program(cfg, tot_slots)
    res = run_bass_kernel_spmd(nc, in_maps, list(range(N_CORES)))
    outs = [r["out"][:cfg.ns] for r in res.results]
    return np.concatenate(outs, axis=0).astype(np.float32)


if __name__ == "__main__":
    cfg = Cfg(N_NODES, D_IN, D_OUT, N_CORES)
    print("cfg", cfg.nb, cfg.nsup, cfg.chunk)


# revision 11
# speedup vs baseline: 1.0210x; 1.0128x over previous
"""GCN (GCNConv + ReLU) message-passing kernel for 8 Trainium2 NeuronCores.

Strategy (dst-sharded graph parallelism):
  - Nodes sharded contiguously across 8 cores (12500 each).
  - Each core computes h'_c = dinv_c * (x_c @ W) for its shard (PE matmul,
    bf16) in 4 row-chunks; each chunk is AllGathered as soon as it is
    written, so per-edge gathers for chunk g start ~80us into the kernel
    instead of waiting for the full exchange.
  - Edges are grouped by (dst core). Per core, edges are laid out in
    (super-block, chunk-group, dst-block) sections padded to the exact
    cross-core max (gather runs 128-padded); per-edge messages are fetched
    with SWDGE dma_gather (int16 indices into 25000-row chunk-group
    buffers).  SWDGE descriptor generation (~9 ns/edge, serial on the Pool
    engine) is the roofline; everything else hides under it.
  - Segment-sum over dst via TensorE: one-hot selectors (VectorE is_equal of
    windowed dst codes 128*block_in_super+lane against per-block iota tiles)
    contracted with gathered message tiles, accumulating in PSUM per 128-dst
    block.  Tiles straddling two blocks are simply visited by both blocks'
    selectors (the fp16 dst code windows disambiguate).
  - Epilogue fuses (psum * dinv[d]) + (dinv[d]^2*h[d] + b), then ReLU.

Host-side work is limited to integer index preprocessing (edge bucketing,
section offsets) and layout/dtype staging; all floating-point math runs on
device.
"""

import math
import sys

import numpy as np

sys.path.insert(0, "/opt/trn_rl_repo")

import ml_dtypes  # noqa: E402

import concourse.bass as bass  # noqa: E402,F401
import concourse.bacc as bacc  # noqa: E402
import concourse.mybir as mybir  # noqa: E402
from concourse import tile  # noqa: E402
from concourse.bass_utils import run_bass_kernel_spmd  # noqa: E402

BF16 = ml_dtypes.bfloat16

# ----- problem constants (hardcoded; kernel.py must be self-contained) -----
N_NODES = 100000
D_IN = 256
D_OUT = 128
N_CORES = 8
G = 4  # AllGather chunks per core == gather index groups


class Cfg:
    """Static, per-compile configuration (identical across cores)."""

    def __init__(self, n_nodes, d_in, d_out, n_cores, idx_range=None,
                 blocks_per_super=8):
        assert n_nodes % n_cores == 0
        self.n_nodes = n_nodes
        self.d_in = d_in
        self.d_out = d_out
        assert d_out == 128, "kernel assumes 128 output features"
        assert d_in % 128 == 0
        self.kchunks = d_in // 128
        self.n_cores = n_cores
        self.ns = n_nodes // n_cores          # nodes per core
        assert self.ns % G == 0
        self.chunk = self.ns // G             # h' rows per AllGather chunk
        self.grp_rows = self.chunk * n_cores  # rows per gathered group buffer
        assert self.grp_rows <= 32768, "int16 gather index range"
        self.nb = math.ceil(self.ns / 128)    # dst blocks per core
        self.bs = blocks_per_super
        self.nsup = math.ceil(self.nb / self.bs)
        self.ns_pad = self.nb * 128
        # filled by preprocessing (shared across cores):
        self.L16 = None        # [nb, G] 16-granular padded section sizes
        self.run_len = None    # [nsup, G] 128-granular run lengths
        self.sec_off = None    # {(b, g): global slot offset}
        self.sup_off = None    # [nsup+1] global slot offset of each super
        self.tot_slots = None

    def blocks_of_super(self, s):
        return range(s * self.bs, min((s + 1) * self.bs, self.nb))


def preprocess(x, edge_index, W, b, cfg: Cfg):
    """Integer/layout-only host prep. Returns per-core input dicts."""
    ns, chunk = cfg.ns, cfg.chunk
    src = np.asarray(edge_index[0], dtype=np.int64)
    dst = np.asarray(edge_index[1], dtype=np.int64)
    x = np.asarray(x, dtype=np.float32)
    W = np.asarray(W, dtype=np.float32)
    b = np.asarray(b, dtype=np.float32)

    core_of = dst // ns
    order = np.argsort(core_of, kind="stable")
    src, dst = src[order], dst[order]
    core_bounds = np.searchsorted(core_of[order], np.arange(cfg.n_cores + 1))

    percore = []
    counts = np.zeros((cfg.n_cores, cfg.nb, G), dtype=np.int64)
    for c in range(cfg.n_cores):
        lo, hi = core_bounds[c], core_bounds[c + 1]
        s_c, d_c = src[lo:hi], dst[lo:hi] - c * ns
        g_c = (s_c % ns) // chunk
        idxv = (s_c // ns) * chunk + (s_c % ns) - g_c * chunk
        blk = d_c // 128
        key = blk * G + g_c
        # sort by (section, src index): ascending gather addresses within a
        # section give the SDMA engines HBM row-buffer locality
        o = np.lexsort((idxv, key))
        s_c, d_c, g_c, idxv, blk = s_c[o], d_c[o], g_c[o], idxv[o], blk[o]
        cnt = np.bincount(key, minlength=cfg.nb * G).reshape(cfg.nb, G)
        counts[c] = cnt
        deg = np.bincount(d_c, minlength=ns) + 1  # + self loop
        percore.append({"d": d_c, "g": g_c, "idx": idxv, "key": key,
                        "deg": deg, "cnt": cnt})

    # Cross-core-uniform padded section sizes (16-granular), runs padded
    # to 128 so each (super, group) gather run starts tile-aligned.
    Lmax = counts.max(axis=0)                       # [nb, G]
    L16 = Lmax.copy()
    cfg.L16 = L16

    run_len = np.zeros((cfg.nsup, G), dtype=np.int64)
    sec_off = {}
    off = 0
    sup_off = [0]
    for s in range(cfg.nsup):
        for g in range(G):
            run_raw = 0
            for bb in cfg.blocks_of_super(s):
                sec_off[(bb, g)] = off + run_raw
                run_raw += int(L16[bb, g])
            rl = ((run_raw + 127) // 128) * 128
            run_len[s, g] = rl
            off += rl
        sup_off.append(off)
    cfg.run_len = run_len
    cfg.sec_off = sec_off
    cfg.sup_off = np.asarray(sup_off, dtype=np.int64)
    tot_slots = int(off)
    cfg.tot_slots = tot_slots
    assert tot_slots % 128 == 0

    in_maps = []
    for c in range(cfg.n_cores):
        pc = percore[c]
        idx_all = np.zeros(tot_slots, dtype=np.int16)
        dst_all = np.full(tot_slots, -1.0, dtype=np.float32)
        # edges sorted by key=(blk, g); place each section's run
        cnt = pc["cnt"]
        starts = np.zeros_like(cnt)
        flat = np.concatenate([[0], np.cumsum(cnt.ravel())])
        for bb in range(cfg.nb):
            s = bb // cfg.bs
            for g in range(G):
                k = bb * G + g
                m = int(cnt[bb, g])
                if m:
                    sl = slice(int(flat[k]), int(flat[k]) + m)
                    o0 = sec_off[(bb, g)]
                    idx_all[o0:o0 + m] = pc["idx"][sl].astype(np.int16)
                    dst_all[o0:o0 + m] = (
                        128 * (bb - s * cfg.bs) + (pc["d"][sl] - bb * 128)
                    ).astype(np.float32)
        # wrap idx into 16 partitions, replicated to 128
        idx_w16 = idx_all.reshape(-1, 16).T.copy()          # [16, tot/16]
        idx_w = np.tile(idx_w16, (8, 1))                     # [128, tot/16]
        dst_w = np.ascontiguousarray(
            dst_all.reshape(-1, 128).T).astype(np.float16)   # [128, tot/128]

        deg_pad = np.ones(cfg.ns_pad, dtype=np.int32)
        deg_pad[:ns] = pc["deg"].astype(np.int32)
        deg_w = np.ascontiguousarray(deg_pad.reshape(cfg.nb, 128).T)

        xs = x[c * ns:(c + 1) * ns]
        xT = np.zeros((cfg.d_in, cfg.ns_pad), dtype=np.float32)
        xT[:, :ns] = xs.T

        in_maps.append({
            "xT": np.ascontiguousarray(xT).astype(BF16),
            "Wm": W.astype(BF16),
            "bias": b.reshape(1, -1).copy(),
            "deg": deg_w,
            "idx": idx_w,
            "dstloc": dst_w,
        })
    return in_maps, tot_slots


def build_program(cfg: Cfg, tot_slots, mock_cc=False, gchunk=8):
    """Builds the SPMD bass program (same NEFF on every core)."""
    fp32 = mybir.dt.float32
    bf16 = mybir.dt.bfloat16
    L16 = cfg.L16
    DO = cfg.d_out
    ns, chunk = cfg.ns, cfg.chunk
    max_idx_per_gather = 1024

    nc = bacc.Bacc("TRN2", target_bir_lowering=False,
                   num_devices=cfg.n_cores, debug=False)

    xT = nc.dram_tensor("xT", [cfg.d_in, cfg.ns_pad], bf16, kind="ExternalInput")
    Wm = nc.dram_tensor("Wm", [cfg.d_in, DO], bf16, kind="ExternalInput")
    bias = nc.dram_tensor("bias", [1, DO], fp32, kind="ExternalInput")
    deg = nc.dram_tensor("deg", [128, cfg.nb], mybir.dt.int32, kind="ExternalInput")
    idx = nc.dram_tensor("idx", [128, tot_slots // 16], mybir.dt.int16,
                         kind="ExternalInput")
    fp16 = mybir.dt.float16
    dstloc = nc.dram_tensor("dstloc", [128, tot_slots // 128], fp16,
                            kind="ExternalInput")
    out = nc.dram_tensor("out", [cfg.ns, DO], fp32, kind="ExternalOutput")

    cc_in = nc.dram_tensor("cc_in", [cfg.ns, DO], bf16)
    cc_out = [nc.dram_tensor(f"cc_out{g}", [cfg.grp_rows, DO], bf16,
                             addr_space="Shared") for g in range(G)]

    # dst-block after which AllGather chunk g can fire
    ag_after_block = [(chunk * (g + 1) + 127) // 128 - 1 for g in range(G)]
    max_sup_tiles = int(cfg.run_len.sum(axis=1).max()) // 128

    from contextlib import ExitStack
    with tile.TileContext(nc) as tc, ExitStack() as ctx:
        const = ctx.enter_context(tc.tile_pool(name="const", bufs=1))
        hpsum = ctx.enter_context(tc.tile_pool(name="hpsum", bufs=2, space="PSUM"))
        htmp = ctx.enter_context(tc.tile_pool(name="htmp", bufs=3))
        msgp = ctx.enter_context(tc.tile_pool(name="msg", bufs=2))
        selp = ctx.enter_context(tc.tile_pool(name="sel", bufs=3))
        pspool = ctx.enter_context(tc.tile_pool(name="ps", bufs=4, space="PSUM"))
        epool = ctx.enter_context(tc.tile_pool(name="ep", bufs=4))

        # ---------------- phase 0: constants ----------------
        W_b = const.tile([128, cfg.kchunks, DO], bf16)
        for k in range(cfg.kchunks):
            nc.sync.dma_start(W_b[:, k, :], Wm[k * 128:(k + 1) * 128, :])

        xb = const.tile([128, cfg.kchunks, cfg.ns_pad], bf16)
        c0_cols = min(cfg.ns_pad, ((chunk + 127) // 128) * 128)
        for k in range(cfg.kchunks):
            nc.sync.dma_start(xb[:, k, :c0_cols],
                              xT[k * 128:(k + 1) * 128, :c0_cols])
        for k in range(cfg.kchunks):
            nc.sync.dma_start(xb[:, k, c0_cols:],
                              xT[k * 128:(k + 1) * 128, c0_cols:])

        deg_i = const.tile([128, cfg.nb], mybir.dt.int32)
        nc.sync.dma_start(deg_i[:, :], deg[:, :])
        deg_f = const.tile([128, cfg.nb], fp32)
        nc.vector.tensor_copy(deg_f[:, :], deg_i[:, :])
        deg_sq = const.tile([128, cfg.nb], fp32)
        nc.scalar.activation(deg_sq[:, :], deg_f[:, :],
                             mybir.ActivationFunctionType.Sqrt)
        dinv = const.tile([128, cfg.nb], fp32)
        nc.vector.reciprocal(dinv[:, :], deg_sq[:, :])
        dinv2 = const.tile([128, cfg.nb], fp32)
        nc.vector.tensor_tensor(dinv2[:, :], dinv[:, :], dinv[:, :],
                                mybir.AluOpType.mult)

        b_row = const.tile([1, DO], fp32)
        nc.sync.dma_start(b_row[:, :], bias[:, :])
        ones_row = const.tile([1, 128], fp32)
        nc.vector.memset(ones_row[:, :], 1.0)
        bt_ps = hpsum.tile([128, DO], fp32, tag="hps")
        nc.tensor.matmul(bt_ps[:, :], ones_row[:, :], b_row[:, :],
                         start=True, stop=True)
        b_tile = const.tile([128, DO], fp32)
        nc.vector.tensor_copy(b_tile[:, :], bt_ps[:, :])

        # per-block-in-super iota tiles: values 128*bb + [0..127]
        iota0 = const.tile([128, 128], fp32)
        nc.gpsimd.iota(iota0[:, :], [[1, 128]], channel_multiplier=0,
                       allow_small_or_imprecise_dtypes=True)
        iota_b = const.tile([128, cfg.bs, 128], fp16)
        for bb in range(cfg.bs):
            nc.vector.tensor_scalar(iota_b[:, bb, :], iota0[:, :],
                                    float(128 * bb), None,
                                    mybir.AluOpType.add)

        sel_tiles_max = 1
        for bb in range(cfg.nb):
            s = bb // cfg.bs
            sup0 = int(cfg.sup_off[s])
            nt = 0
            for g in range(G):
                ln = int(L16[bb, g])
                if ln == 0:
                    continue
                o = cfg.sec_off[(bb, g)]
                nt += (o + ln - 1 - sup0) // 128 - (o - sup0) // 128 + 1
            sel_tiles_max = max(sel_tiles_max, nt)

        # ---------------- phase A: h' = dinv * (x @ W) -------------------
        # h' blocks staged in SBUF; each AllGather chunk written to cc_in
        # with one large DMA (per-block writes pay ~2us HWDGE issue each).
        hstage = const.tile([128, cfg.nb, DO], bf16)
        bnds = sorted({min(cfg.ns, ((chunk * (g + 1) + 127) // 128) * 128)
                       for g in range(G)})
        dma_after_block = {(hi + 127) // 128 - 1: (lo, hi)
                           for lo, hi in zip([0] + bnds[:-1], bnds)}
        for k in range(cfg.nb):
            ph = hpsum.tile([128, DO], fp32, tag="hps")
            for kb in range(cfg.kchunks):
                nc.tensor.matmul(ph[:, :], xb[:, kb, k * 128:(k + 1) * 128],
                                 W_b[:, kb, :],
                                 start=(kb == 0), stop=(kb == cfg.kchunks - 1))
            nc.vector.tensor_scalar(hstage[:, k, :], ph[:, :],
                                    dinv[:, k:k + 1], None,
                                    mybir.AluOpType.mult)
            if k in dma_after_block:
                lo, hi = dma_after_block[k]
                full_hi = (hi // 128) * 128
                if full_hi > lo:
                    nc.sync.dma_start(
                        cc_in[lo:full_hi, :].rearrange(
                            "(t p) d -> p t d", p=128),
                        hstage[:, lo // 128:full_hi // 128, :])
                if hi > full_hi:
                    nc.sync.dma_start(cc_in[full_hi:hi, :],
                                      hstage[:hi - full_hi, hi // 128, :])


        idx_sb = const.tile([128, tot_slots // 16], mybir.dt.int16)
        nc.sync.dma_start(idx_sb[:, :], idx[:, :])
        dst_sb = const.tile([128, tot_slots // 128], fp16)
        nc.sync.dma_start(dst_sb[:, :], dstloc[:, :])

        # ---------------- phase C: gather + segment-sum + epilogue -------
        ag_done = set()

        def dispatch_ag(g):
            ag_done.add(g)
            lo, hi = chunk * g, chunk * (g + 1)
            if cfg.n_cores > 1 and not mock_cc:
                nc.gpsimd.collective_compute(
                    "AllGather",
                    mybir.AluOpType.bypass,
                    replica_groups=[list(range(cfg.n_cores))],
                    ins=[cc_in[lo:hi, :]],
                    outs=[cc_out[g][:, :]],
                )
            else:
                for cpy in range(cfg.n_cores):
                    nc.sync.dma_start(
                        cc_out[g][cpy * chunk:(cpy + 1) * chunk, :],
                        cc_in[lo:hi, :])

        for s in range(cfg.nsup):
            nt_sup = int(cfg.run_len[s].sum()) // 128
            if nt_sup == 0:
                continue
            sup0 = int(cfg.sup_off[s])
            msg = msgp.tile([128, max_sup_tiles, 128], bf16, tag="msg")
            for g in range(G):
                rl = int(cfg.run_len[s, g])
                if rl == 0:
                    continue
                if g not in ag_done:
                    dispatch_ag(g)
                run0 = cfg.sec_off[(s * cfg.bs, g)]  # first block's section
                t0 = (run0 - sup0) // 128
                for c0 in range(0, rl, max_idx_per_gather):
                    n_sg = min(max_idx_per_gather, rl - c0)
                    slot0 = run0 + c0
                    nc.gpsimd.dma_gather(
                        msg[:, t0 + c0 // 128:t0 + (c0 + n_sg) // 128, :],
                        cc_out[g][:, :],
                        idx_sb[:, slot0 // 16:(slot0 + n_sg) // 16],
                        n_sg, n_sg, DO, single_packet=False)

            for bb in cfg.blocks_of_super(s):
                lb = bb - s * cfg.bs
                ps = pspool.tile([128, DO], fp32, tag="ps")
                # (tile, g) pairs this block's sections touch
                tiles = []
                for g in range(G):
                    o = cfg.sec_off[(bb, g)]
                    ln = int(L16[bb, g])
                    if ln == 0:
                        continue
                    ta = (o - sup0) // 128
                    tz = (o + ln - 1 - sup0) // 128
                    tiles.extend(range(ta, tz + 1))
                tiles = sorted(set(tiles))
                if not tiles:
                    t1 = epool.tile([128, DO], fp32, tag="t1")
                    nc.vector.scalar_tensor_tensor(
                        t1[:, :], hstage[:, bb, :], dinv[:, bb:bb + 1],
                        b_tile[:, :],
                        mybir.AluOpType.mult, mybir.AluOpType.add)
                    t2 = epool.tile([128, DO], fp32, tag="t2")
                    nc.scalar.activation(t2[:, :], t1[:, :],
                                         mybir.ActivationFunctionType.Relu)
                    rows = min(128, cfg.ns - bb * 128)
                    nc.sync.dma_start(out[bb * 128:bb * 128 + rows, :],
                                      t2[:rows, :])
                    continue
                sel = selp.tile([128, sel_tiles_max, 128], bf16, tag="sel")
                for j, t in enumerate(tiles):
                    d_ap = dst_sb[:, sup0 // 128 + t:sup0 // 128 + t + 1]
                    d_b = d_ap.broadcast_to((128, 128))
                    nc.vector.tensor_tensor(sel[:, j, :], d_b,
                                            iota_b[:, lb, :],
                                            mybir.AluOpType.is_equal)
                    nc.tensor.matmul(
                        ps[:, :], sel[:, j, :], msg[:, t, :],
                        start=(j == 0), stop=(j == len(tiles) - 1))
                t0 = epool.tile([128, DO], fp32, tag="t0")
                nc.vector.scalar_tensor_tensor(
                    t0[:, :], hstage[:, bb, :], dinv[:, bb:bb + 1],
                    b_tile[:, :],
                    mybir.AluOpType.mult, mybir.AluOpType.add)
                t1 = epool.tile([128, DO], fp32, tag="t1")
                nc.vector.scalar_tensor_tensor(
                    t1[:, :], ps[:, :], dinv[:, bb:bb + 1], t0[:, :],
                    mybir.AluOpType.mult, mybir.AluOpType.add)
                t2 = epool.tile([128, DO], fp32, tag="t2")
                nc.scalar.activation(t2[:, :], t1[:, :],
                                     mybir.ActivationFunctionType.Relu)
                rows = min(128, cfg.ns - bb * 128)
                nc.sync.dma_start(out[bb * 128:bb * 128 + rows, :],
                                  t2[:rows, :])

    nc.compile()
    return nc


def kernel(x, edge_index, W, b):
    cfg = Cfg(N_NODES, D_IN, D_OUT, N_CORES)
    in_maps, tot_slots = preprocess(x, edge_index, W, b, cfg)
    nc = build_program(cfg, tot_slots)
    res = run_bass_kernel_spmd(nc, in_maps, list(range(N_CORES)))
    outs = [r["out"][:cfg.ns] for r in res.results]
    return np.concatenate(outs, axis=0).astype(np.float32)


if __name__ == "__main__":
    cfg = Cfg(N_NODES, D_IN, D_OUT, N_CORES)
    print("cfg", cfg.nb, cfg.nsup, cfg.chunk)
